# revision 6
# baseline (speedup 1.0000x reference)
"""Trainium2 Bass kernel for nn_Att_AdaIn (B=4, C=256, H=W=64 attention block).

Sharding: 8 cores = 4 batches x 2 query-halves. Each core holds full C x C
weights, the full key/value source y[b] ([256, 4096]), and its own query
slice x[b][:, half] ([256, 2048]); it computes the full attention output for
its 2048 queries. Host gathers the 8 [256, 2048] results.

Per-core algorithm (all layouts chosen so no on-chip transpose is needed):
  q  = WqT.T @ x + bq            [c_out, l]   (c_out on partitions)
  k  = WkT.T @ y + bk            [c_out, j]
  vTa= y.T @ WvTa + bva          [j, 257]     (j on partitions; col 256 == 1
                                              via zero weight column + bias 1,
                                              giving softmax denominators for free)
  ST = k.T @ q                   [j, l]       (transposed attention scores)
  E  = exp(ST / sqrt(C))         (no max-subtraction: logits ~ N(0,1), fp32-safe)
  zA = vTa.T @ E                 [257, l]     rows 0..255 = unnormalized z,
                                              row 256 = softmax denominator
  t  = WoT.T @ z                 [o, l]
  out= t * (1/denom) + bo + x

The matmul dtype is selectable: float32 (4 cyc/row), float32r (fast fp32
path, 1 cyc/row), bfloat16 (1 cyc/row). For float32r/bfloat16 every tile
feeding a matmul is typed in that dtype (BIR requires producers to round).
"""

import os
import sys

for _p in ("/root/.axon_site", "/root/.axon_site/_ro/trn_rl_repo", "/opt/trn_rl_repo"):
    if os.path.isdir(_p) and _p not in sys.path:
        sys.path.append(_p)

import numpy as np

import concourse.bass as bass
from concourse import bacc, mybir, tile
from concourse import bass_utils

B, C, H, W = 4, 256, 64, 64
N = H * W          # 4096 pixels
NQ = N // 2        # 2048 queries per core
P = 128
A = C // P         # 2 channel chunks
LT = 512           # l-tile (query) width
NLT = NQ // LT     # 4 l-tiles
JC = N // P        # 32 key chunks
SCALE = 1.0 / np.sqrt(np.float32(C))  # 1/16
CP = C + 2         # v columns padded: [Wv.T | ones-col | zero-pad] (even for f32r)

MATMUL_DT = os.environ.get("ATT_MATMUL_DT", "float32r")


def build_nc(matmul_dt_name: str = MATMUL_DT):
    mdt = getattr(mybir.dt, matmul_dt_name)
    f32 = mybir.dt.float32
    is_bf16 = mdt == mybir.dt.bfloat16

    nc = bacc.Bacc("TRN2", target_bir_lowering=False, debug=False)

    x_d = nc.dram_tensor("x", [C, NQ], mdt, kind="ExternalInput").ap()
    y_d = nc.dram_tensor("y", [C, N], mdt, kind="ExternalInput").ap()
    wqT_d = nc.dram_tensor("wqT", [C, C], mdt, kind="ExternalInput").ap()
    wkT_d = nc.dram_tensor("wkT", [C, C], mdt, kind="ExternalInput").ap()
    wvTa_d = nc.dram_tensor("wvTa", [C, CP], mdt, kind="ExternalInput").ap()
    woT_d = nc.dram_tensor("woT", [C, C], mdt, kind="ExternalInput").ap()
    if is_bf16:
        xres_d = nc.dram_tensor("xres", [C, NQ], f32, kind="ExternalInput").ap()
    bq_d = nc.dram_tensor("bq", [C], f32, kind="ExternalInput").ap()
    bk_d = nc.dram_tensor("bk", [C], f32, kind="ExternalInput").ap()
    bva_d = nc.dram_tensor("bva", [CP], f32, kind="ExternalInput").ap()
    bo_d = nc.dram_tensor("bo", [C], f32, kind="ExternalInput").ap()
    out_d = nc.dram_tensor("out", [C, NQ], f32, kind="ExternalOutput").ap()

    xr = x_d.rearrange("(a p) n -> p a n", p=P)
    yr = y_d.rearrange("(a p) n -> p a n", p=P)
    outr = out_d.rearrange("(a p) n -> p a n", p=P)

    with tile.TileContext(nc) as tc:
        with (
            tc.tile_pool(name="const", bufs=1) as const,
            tc.tile_pool(name="epool", bufs=4) as epool,
            tc.tile_pool(name="zpool", bufs=2) as zpool,
            tc.tile_pool(name="opool", bufs=2) as opool,
            tc.tile_pool(name="rpool", bufs=2) as rpool,
            tc.tile_pool(name="ps_st", bufs=2, space="PSUM") as ps_st,
            tc.tile_pool(name="ps_zq", bufs=1, space="PSUM") as ps_zq,
            tc.tile_pool(name="ps_den", bufs=1, space="PSUM") as ps_den,
            tc.tile_pool(name="ps_misc", bufs=1, space="PSUM") as ps_misc,
            tc.tile_pool(name="ps_z2", bufs=2, space="PSUM") as ps_z2,
        ):
            # ---- persistent SBUF tensors ----
            x_sb = const.tile([P, A, NQ], mdt)
            y_sb = const.tile([P, A, N], mdt)
            wq_sb = const.tile([P, A, C], mdt)
            wk_sb = const.tile([P, A, C], mdt)
            wv_sb = const.tile([P, A, CP], mdt)
            wo_sb = const.tile([P, A, C], mdt)
            bq_sb = const.tile([P, A], f32)
            bk_sb = const.tile([P, A], f32)
            bo_sb = const.tile([P, A], f32)
            bva_bc = const.tile([P, CP], f32)
            ones_sb = const.tile([1, P], f32)
            q_sb = const.tile([P, A, NQ], mdt)
            k_sb = const.tile([P, A, N], mdt)
            vTa_sb = const.tile([P, JC, CP], mdt)
            if is_bf16:
                xres_sb = const.tile([P, A, NQ], f32)
            else:
                xres_sb = x_sb.bitcast(f32)

            # ---- loads ----
            nc.sync.dma_start(out=x_sb, in_=xr)
            nc.sync.dma_start(out=y_sb, in_=yr)
            nc.sync.dma_start(out=wq_sb, in_=wqT_d.rearrange("(a p) o -> p a o", p=P))
            nc.sync.dma_start(out=wk_sb, in_=wkT_d.rearrange("(a p) o -> p a o", p=P))
            nc.sync.dma_start(out=wv_sb, in_=wvTa_d.rearrange("(a p) o -> p a o", p=P))
            nc.sync.dma_start(out=wo_sb, in_=woT_d.rearrange("(a p) o -> p a o", p=P))
            nc.sync.dma_start(out=bq_sb, in_=bq_d.rearrange("(a p) -> p a", p=P))
            nc.sync.dma_start(out=bk_sb, in_=bk_d.rearrange("(a p) -> p a", p=P))
            nc.sync.dma_start(out=bo_sb, in_=bo_d.rearrange("(a p) -> p a", p=P))
            bva_bcast_ap = bass.AP(
                tensor=bva_d.tensor,
                offset=bva_d.offset,
                ap=[[0, P], list(bva_d.ap[0])],
            )
            nc.sync.dma_start(out=bva_bc, in_=bva_bcast_ap)
            nc.vector.memset(ones_sb, 1.0)
            if is_bf16:
                nc.sync.dma_start(
                    out=xres_sb, in_=xres_d.rearrange("(a p) n -> p a n", p=P)
                )

            # ---- projections ----
            # q[o, l] = sum_c WqT[c, o] x[c, l] + bq[o]
            for och in range(A):
                for lt in range(NLT):
                    ps = ps_st.tile([P, LT], f32, tag="st")
                    for a in range(A):
                        nc.tensor.matmul(
                            ps,
                            wq_sb[:, a, och * P:(och + 1) * P],
                            x_sb[:, a, lt * LT:(lt + 1) * LT],
                            start=(a == 0),
                            stop=(a == A - 1),
                        )
                    nc.scalar.activation(
                        out=q_sb[:, och, lt * LT:(lt + 1) * LT],
                        in_=ps,
                        func=mybir.ActivationFunctionType.Identity,
                        bias=bq_sb[:, och:och + 1],
                    )
            # k[o, j] = sum_c WkT[c, o] y[c, j] + bk[o]
            for och in range(A):
                for jt in range(N // LT):
                    ps = ps_st.tile([P, LT], f32, tag="st")
                    for a in range(A):
                        nc.tensor.matmul(
                            ps,
                            wk_sb[:, a, och * P:(och + 1) * P],
                            y_sb[:, a, jt * LT:(jt + 1) * LT],
                            start=(a == 0),
                            stop=(a == A - 1),
                        )
                    nc.scalar.activation(
                        out=k_sb[:, och, jt * LT:(jt + 1) * LT],
                        in_=ps,
                        func=mybir.ActivationFunctionType.Identity,
                        bias=bk_sb[:, och:och + 1],
                    )
            # vTa[j, o] = sum_c y[c, j] WvTa[c, o] + bva[o]   (o in 0..257)
            for jc in range(JC):
                ps = ps_st.tile([P, CP], f32, tag="st")
                for a in range(A):
                    nc.tensor.matmul(
                        ps,
                        y_sb[:, a, jc * P:(jc + 1) * P],
                        wv_sb[:, a, :],
                        start=(a == 0),
                        stop=(a == A - 1),
                    )
                nc.vector.tensor_add(out=ps, in0=ps, in1=bva_bc)
                nc.scalar.activation(
                    out=vTa_sb[:, jc, :],
                    in_=ps,
                    func=mybir.ActivationFunctionType.Copy,
                )

            # ---- attention, l-tile at a time ----
            for lt in range(NLT):
                lsl = slice(lt * LT, (lt + 1) * LT)
                zq0 = ps_zq.tile([P, LT], f32, tag="zq0")
                zq1 = ps_zq.tile([P, LT], f32, tag="zq1")
                zq = (zq0, zq1)
                den = ps_den.tile([2, LT], f32)
                for jc in range(JC):
                    st = ps_st.tile([P, LT], f32, tag="st")
                    for a in range(A):
                        nc.tensor.matmul(
                            st,
                            k_sb[:, a, jc * P:(jc + 1) * P],
                            q_sb[:, a, lsl],
                            start=(a == 0),
                            stop=(a == A - 1),
                        )
                    e_sb = epool.tile([P, LT], mdt)
                    nc.scalar.activation(
                        out=e_sb,
                        in_=st,
                        func=mybir.ActivationFunctionType.Exp,
                        scale=float(SCALE),
                    )
                    for m in range(A):
                        nc.tensor.matmul(
                            zq[m],
                            vTa_sb[:, jc, m * P:(m + 1) * P],
                            e_sb,
                            start=(jc == 0),
                            stop=(jc == JC - 1),
                        )
                    nc.tensor.matmul(
                        den,
                        vTa_sb[:, jc, C:CP],
                        e_sb,
                        start=(jc == 0),
                        stop=(jc == JC - 1),
                    )

                # reciprocal of denominators, broadcast across partitions via PE
                r_sb = rpool.tile([1, LT], f32, tag="r")
                nc.vector.reciprocal(out=r_sb, in_=den[0:1, :])
                rbc_ps = ps_misc.tile([P, LT], f32)
                nc.tensor.matmul(rbc_ps, ones_sb, r_sb, start=True, stop=True)
                rbc_sb = rpool.tile([P, LT], f32, tag="rbc")
                nc.scalar.activation(
                    out=rbc_sb, in_=rbc_ps, func=mybir.ActivationFunctionType.Copy
                )

                # evict unnormalized z to SBUF for the output projection
                z_sb = zpool.tile([P, A, LT], mdt)
                for m in range(A):
                    nc.scalar.activation(
                        out=z_sb[:, m, :],
                        in_=zq[m],
                        func=mybir.ActivationFunctionType.Copy,
                    )

                # t = WoT.T @ z ; out = t * r + bo + x
                for och in range(A):
                    z2 = ps_z2.tile([P, LT], f32)
                    for a in range(A):
                        nc.tensor.matmul(
                            z2,
                            wo_sb[:, a, och * P:(och + 1) * P],
                            z_sb[:, a, :],
                            start=(a == 0),
                            stop=(a == A - 1),
                        )
                    o_sb = opool.tile([P, LT], f32)
                    nc.vector.tensor_mul(out=o_sb, in0=z2, in1=rbc_sb)
                    nc.vector.tensor_scalar_add(
                        out=o_sb, in0=o_sb, scalar1=bo_sb[:, och:och + 1]
                    )
                    nc.vector.tensor_add(out=o_sb, in0=o_sb, in1=xres_sb[:, och, lsl])
                    nc.sync.dma_start(out=outr[:, och, lsl], in_=o_sb)

    nc.compile()
    return nc


_NC_CACHE = {}


def _get_nc(matmul_dt_name: str = MATMUL_DT):
    if matmul_dt_name not in _NC_CACHE:
        _NC_CACHE[matmul_dt_name] = build_nc(matmul_dt_name)
    return _NC_CACHE[matmul_dt_name]


def make_in_maps(x, y, Wq, bq, Wk, bk, Wv, bv, Wo, bo, matmul_dt_name: str = MATMUL_DT):
    f32 = np.float32
    if matmul_dt_name == "bfloat16":
        import ml_dtypes

        mnp = ml_dtypes.bfloat16
    else:
        mnp = np.float32
    xf = np.asarray(x, f32).reshape(B, C, N)
    yf = np.asarray(y, f32).reshape(B, C, N)
    wqT = np.ascontiguousarray(np.asarray(Wq, f32).T).astype(mnp)
    wkT = np.ascontiguousarray(np.asarray(Wk, f32).T).astype(mnp)
    woT = np.ascontiguousarray(np.asarray(Wo, f32).T).astype(mnp)
    wvTa = np.concatenate(
        [np.asarray(Wv, f32).T, np.zeros((C, 2), f32)], axis=1
    ).astype(mnp)
    bva = np.concatenate([np.asarray(bv, f32), np.asarray([1.0, 0.0], f32)])
    bq = np.asarray(bq, f32)
    bk = np.asarray(bk, f32)
    bo = np.asarray(bo, f32)
    ym = yf.astype(mnp) if mnp is not np.float32 else yf
    in_maps = []
    for core in range(8):
        b, h = divmod(core, 2)
        xs = np.ascontiguousarray(xf[b][:, h * NQ:(h + 1) * NQ])
        m = {
            "x": xs.astype(mnp) if mnp is not np.float32 else xs,
            "y": ym[b],
            "wqT": wqT, "wkT": wkT, "wvTa": wvTa, "woT": woT,
            "bq": bq, "bk": bk, "bva": bva, "bo": bo,
        }
        if matmul_dt_name == "bfloat16":
            m["xres"] = xs
        in_maps.append(m)
    return in_maps


def kernel(x, y, Wq, bq, Wk, bk, Wv, bv, Wo, bo):
    nc = _get_nc()
    in_maps = make_in_maps(x, y, Wq, bq, Wk, bk, Wv, bv, Wo, bo)
    res = bass_utils.run_bass_kernel_spmd(nc, in_maps, core_ids=list(range(8)))
    out = np.empty((B, C, N), np.float32)
    for core in range(8):
        b, h = divmod(core, 2)
        out[b][:, h * NQ:(h + 1) * NQ] = res.results[core]["out"]
    return out.reshape(B, C, H, W)
